# revision 8
# baseline (speedup 1.0000x reference)
"""DGCNN (3x DynamicEdgeConv + head MLP) Trainium2 Bass kernel.

Data-parallel over B=8 graphs: one NeuronCore per graph (N=2048 points,
K=30 neighbors). Per core:
  - kNN: negkey[i,j] = 2*f_i.f_j - |f_j|^2 via one augmented PE matmul
    ([f;1]^T @ [2f;-d2]); exact top-30 per row via DVE max8/max_index/
    match_replace rounds (ranking by negkey == ranking by -dist).
  - EdgeConv MLP decomposition: relu(e@wa+ba) = relu(u_i + v_j) with
    u = x@(wa_top-wa_bot)+ba, v = x@wa_bot.  v lives in HBM row-major;
    neighbors fetched with indirect DMA (CCE-add fuses the +u_i).
  - Edge features transposed to channel-major in k-slot pairs on PE,
    relu on ACT (psum->sbuf), second MLP layer (@wb) on PE, max over
    K on DVE strided-view reduce, +bb folded into one ACT pass.
  - Head MLP computed channel-major (output transpose is free on host).
All compute fp32.
"""

import sys

sys.path.insert(0, "/opt/trn_rl_repo")

import numpy as np

import concourse.bass as bass
import concourse.bacc as bacc
import concourse.mybir as mybir
import concourse.tile as tile
from concourse.bass_utils import run_bass_kernel_spmd
from concourse.masks import make_identity

F32 = mybir.dt.float32
U32 = mybir.dt.uint32
AF = mybir.ActivationFunctionType
ALU = mybir.AluOpType

P = 128
K = 30
KP = 32           # padded neighbor count (pad slots duplicate slot 0)
NEG_BIG = -3.0e38


def _bc_free(ap, count, inner):
    """Insert a 0-stride broadcast dim: [P, inner] -> [P, count, inner]."""
    return bass.AP(ap.tensor, ap.offset, [list(ap.ap[0]), [0, count], [1, inner]])


def _edge_conv(tc, pools, d, x_cm, wu_t, wv_t, ba_t, wb_t, bb_t, x_next,
               v_hbm, ones_t, ident, N, gsem):
    """One DynamicEdgeConv. x_cm: [d+1, N] sbuf tile (row d == 1.0).
    Writes x_next[0:64, :] (and caller-visible row 64 must be set by caller).
    """
    nc = tc.nc
    NT = N // P
    sb_work, sb_small, ps_s = pools["work"], pools["small"], pools["ps_s"]
    ps_t_pool, ps_h2_pool = pools["ps_t"], pools["ps_h2"]
    da = 32 if d <= 32 else 64   # partition row holding the aug (1 / -d2) entry

    # ---- phase A: u, v, d2, aug_r ----
    sq = sb_work.tile([64, N], F32, tag="sq", bufs=1)
    nc.scalar.activation(sq[0:d, :], x_cm[0:d, :], AF.Square)
    augr = sb_work.tile([65, N], F32, tag="augr", bufs=1)
    if d < da:
        nc.vector.memset(augr[0:da + 1, :], 0.0)
    # rows 0:d = 2*x
    nc.scalar.activation(augr[0:d, :], x_cm[0:d, :], AF.Copy, scale=2.0)
    jblk = min(512, N)
    for jc in range(N // jblk):
        ps_d2 = ps_s.tile([1, jblk], F32, tag="ps64")
        nc.tensor.matmul(ps_d2[:], ones_t[0:d, :], sq[0:d, bass.ts(jc, jblk)],
                         start=True, stop=True)
        # row da = -d2
        nc.scalar.activation(augr[da:da + 1, bass.ts(jc, jblk)], ps_d2[:],
                             AF.Copy, scale=-1.0)

    u_rm = sb_work.tile([P, NT * 64], F32, tag="u_rm", bufs=1)
    v_rm = sb_work.tile([P, NT * 64], F32, tag="v_rm", bufs=1)
    for it in range(NT):
        ps_u = ps_s.tile([P, 64], F32, tag="ps64")
        nc.tensor.matmul(ps_u[:], x_cm[0:d, bass.ts(it, P)], wu_t[0:d, :],
                         start=True, stop=True)
        nc.vector.tensor_add(u_rm[:, bass.ts(it, 64)], ps_u[:], ba_t[:])
        ps_v = ps_s.tile([P, 64], F32, tag="ps64")
        nc.tensor.matmul(ps_v[:], x_cm[0:d, bass.ts(it, P)], wv_t[0:d, :],
                         start=True, stop=True)
        nc.scalar.activation(v_rm[:, bass.ts(it, 64)], ps_v[:], AF.Copy)
    # v -> HBM (row-major [N, 64]) as the gather source
    v_hbm_view = v_hbm[:].rearrange("(t p) c -> p t c", p=P)
    nc.sync.dma_start(v_hbm_view, v_rm[:].rearrange("p (t c) -> p t c", c=64))

    # ---- phase B: per i-tile ----
    for it in range(NT):
        # dist keys: negkey[i, j] = 2 f_i.f_j - d2_j
        negkey = sb_work.tile([P, N], F32, tag="negkey")
        for jc in range(N // jblk):
            ps_nk = ps_s.tile([P, jblk], F32, tag="ps512")
            nc.tensor.matmul(ps_nk[:], x_cm[0:da + 1, bass.ts(it, P)],
                             augr[0:da + 1, bass.ts(jc, jblk)],
                             start=True, stop=True)
            nc.scalar.activation(negkey[:, bass.ts(jc, jblk)], ps_nk[:], AF.Copy)

        # top-30 (+2 dup pads) via 4 rounds of max8
        idx_t = sb_small.tile([P, KP], U32, tag="idx")
        for r in range(4):
            m8 = sb_small.tile([P, 8], F32, tag="m8")
            nc.vector.max(m8[:], negkey[:])
            nc.vector.max_index(idx_t[:, r * 8:(r + 1) * 8], m8[:], negkey[:])
            if r < 3:
                nc.vector.match_replace(negkey[:], m8[:], negkey[:], NEG_BIG)
        # pad slots 30,31 <- slot 0 (self), so max-aggregation is unaffected
        nc.vector.tensor_copy(idx_t[:, 30:31], idx_t[:, 0:1])
        nc.vector.tensor_copy(idx_t[:, 31:32], idx_t[:, 0:1])

        # edge pre-activations: etile[i, k, c] = u_i[c] (+ v_j[c] via CCE add)
        etile = sb_work.tile([P, KP * 64], F32, tag="etile")
        u_slice = u_rm[:, bass.ts(it, 64)]
        nc.scalar.activation(etile[:], _bc_free(u_slice, KP, 64), AF.Copy)
        ev = etile[:].rearrange("p (k c) -> p k c", c=64)
        with tc.tile_critical():
            for k in range(KP):
                nc.gpsimd.indirect_dma_start(
                    out=ev[:, k, :], out_offset=None,
                    in_=v_hbm[:],
                    in_offset=bass.IndirectOffsetOnAxis(
                        ap=idx_t[:, k:k + 1], axis=0),
                    compute_op=ALU.add).then_inc(gsem[0], 16)
            gsem[1] += 16 * KP
            nc.gpsimd.wait_ge(gsem[0], gsem[1])

        # k-pair transpose -> relu -> @wb -> segment-max
        acc = None
        for g in range(KP // 8):
            ps_t = ps_t_pool.tile([P, 512], F32, tag="ps_t")
            for t in range(4):
                tt = g * 4 + t
                nc.tensor.transpose(ps_t[:, bass.ts(t, P)],
                                    etile[:, bass.ts(tt, P)], ident[:])
            rhs = sb_small.tile([P, 512], F32, tag="rhs")
            nc.scalar.activation(rhs[:], ps_t[:], AF.Relu)
            hp = ps_h2_pool.tile([64, 1024], F32, tag="ps_h2")
            nc.tensor.matmul(hp[:, 0:512], wb_t[0:64, :], rhs[0:64, :],
                             start=True, stop=True)
            nc.tensor.matmul(hp[:, 512:1024], wb_t[64:128, :], rhs[64:128, :],
                             start=True, stop=True)
            # view [64, 128 i, 2 parity, 4 t] -> reduce XY -> [64, 128]
            hview = bass.AP(hp.tensor, 0,
                            [[hp[:].ap[0][0], 64], [1, P], [512, 2], [P, 4]])
            if g == 0:
                acc = sb_small.tile([64, P], F32, tag="seg_acc")
                nc.vector.tensor_reduce(acc[:], hview, mybir.AxisListType.XY,
                                        ALU.max)
            else:
                tmp = sb_small.tile([64, P], F32, tag="seg_tmp")
                nc.vector.tensor_reduce(tmp[:], hview, mybir.AxisListType.XY,
                                        ALU.max)
                dst = (x_next[0:64, bass.ts(it, P)] if g == KP // 8 - 1
                       else acc[:])
                nc.vector.tensor_tensor(dst, acc[:], tmp[:], ALU.max)

    # +bb (per-channel bias) in place
    nc.scalar.activation(x_next[0:64, :], x_next[0:64, :], AF.Identity,
                         bias=bb_t[:, 0:1])
    # aug ones row for the next conv's lhsT
    nc.vector.memset(x_next[64:65, :], 1.0)


def _head(tc, pools, x1, x2, x3, hw1_t, hb1_t, hw2_t, hb2_t, hw3_t, hb3_t,
          out_sb, N):
    """Head MLP, channel-major: relu(feat@hw1+hb1) -> relu(@hw2+hb2) -> @hw3+hb3."""
    nc = tc.nc
    sb_head, ps_s = pools["head"], pools["ps_s"]
    xs = [x1, x2, x3]
    blk = min(512, N)
    for ib in range(N // blk):
        isl = bass.ts(ib, blk)
        h1 = sb_head.tile([P, 8 * blk], F32, tag="h1")
        for c in range(8):
            ps = ps_s.tile([P, blk], F32, tag="ps512")
            for kc in range(3):
                nc.tensor.matmul(ps[:],
                                 hw1_t[:, kc * 1024 + c * P:kc * 1024 + (c + 1) * P],
                                 xs[kc][0:64, isl],
                                 start=(kc == 0), stop=(kc == 2))
            nc.scalar.activation(h1[:, bass.ts(c, blk)], ps[:], AF.Relu,
                                 bias=hb1_t[:, c:c + 1])
        h2 = sb_head.tile([P, 4 * blk], F32, tag="h2")
        for c in range(4):
            ps = ps_s.tile([P, blk], F32, tag="ps512")
            for kc in range(8):
                nc.tensor.matmul(ps[:],
                                 hw2_t[:, kc * 512 + c * P:kc * 512 + (c + 1) * P],
                                 h1[:, bass.ts(kc, blk)],
                                 start=(kc == 0), stop=(kc == 7))
            nc.scalar.activation(h2[:, bass.ts(c, blk)], ps[:], AF.Relu,
                                 bias=hb2_t[:, c:c + 1])
        ps = ps_s.tile([P, blk], F32, tag="ps512")
        for kc in range(4):
            nc.tensor.matmul(ps[:], hw3_t[:, bass.ts(kc, P)],
                             h2[:, bass.ts(kc, blk)],
                             start=(kc == 0), stop=(kc == 3))
        nc.scalar.activation(out_sb[:, isl], ps[:], AF.Identity,
                             bias=hb3_t[:, 0:1])


def build_program(N):
    nc = bacc.Bacc("TRN2", target_bir_lowering=False, debug=False)
    NT = N // P

    x0cm_d = nc.dram_tensor("x0cm", [6, N], F32, kind="ExternalInput")
    w_d = {}
    for l, d in ((1, 6), (2, 64), (3, 64)):
        w_d[f"wu{l}"] = nc.dram_tensor(f"wu{l}", [d, 64], F32, kind="ExternalInput")
        w_d[f"wv{l}"] = nc.dram_tensor(f"wv{l}", [d, 64], F32, kind="ExternalInput")
        w_d[f"ba{l}"] = nc.dram_tensor(f"ba{l}", [P, 64], F32, kind="ExternalInput")
        w_d[f"wb{l}"] = nc.dram_tensor(f"wb{l}", [P, 64], F32, kind="ExternalInput")
        w_d[f"bb{l}"] = nc.dram_tensor(f"bb{l}", [64, 1], F32, kind="ExternalInput")
    w_d["hw1"] = nc.dram_tensor("hw1", [64, 3 * 1024], F32, kind="ExternalInput")
    w_d["hb1"] = nc.dram_tensor("hb1", [P, 8], F32, kind="ExternalInput")
    w_d["hw2"] = nc.dram_tensor("hw2", [P, 8 * 512], F32, kind="ExternalInput")
    w_d["hb2"] = nc.dram_tensor("hb2", [P, 4], F32, kind="ExternalInput")
    w_d["hw3"] = nc.dram_tensor("hw3", [P, 4 * P], F32, kind="ExternalInput")
    w_d["hb3"] = nc.dram_tensor("hb3", [P, 1], F32, kind="ExternalInput")
    out_d = nc.dram_tensor("out_cm", [P, N], F32, kind="ExternalOutput")
    v_hbm = [nc.dram_tensor(f"vscratch{l}", [N, 64], F32) for l in (1, 2, 3)]

    with tile.TileContext(nc) as tc:
        with tc.tile_pool(name="persist", bufs=1) as sb_p, \
             tc.tile_pool(name="work", bufs=2) as sb_work, \
             tc.tile_pool(name="small", bufs=3) as sb_small, \
             tc.tile_pool(name="head", bufs=1) as sb_head, \
             tc.tile_pool(name="ps_s", bufs=2, space="PSUM") as ps_s, \
             tc.tile_pool(name="ps_t", bufs=2, space="PSUM") as ps_t_pool, \
             tc.tile_pool(name="ps_h2", bufs=1, space="PSUM") as ps_h2_pool:
            pools = {"work": sb_work, "small": sb_small, "head": sb_head,
                     "ps_s": ps_s, "ps_t": ps_t_pool, "ps_h2": ps_h2_pool}

            ident = sb_p.tile([P, P], F32, tag="ident")
            make_identity(nc, ident[:])
            ones_t = sb_p.tile([64, 1], F32, tag="ones")
            nc.vector.memset(ones_t[:], 1.0)

            def load(name, shape, tag):
                t = sb_p.tile(shape, F32, tag=tag)
                nc.sync.dma_start(t[:], w_d[name][:])
                return t

            wt = {}
            for l, d in ((1, 6), (2, 64), (3, 64)):
                wt[f"wu{l}"] = load(f"wu{l}", [d, 64], f"wu{l}")
                wt[f"wv{l}"] = load(f"wv{l}", [d, 64], f"wv{l}")
                wt[f"ba{l}"] = load(f"ba{l}", [P, 64], f"ba{l}")
                wt[f"wb{l}"] = load(f"wb{l}", [P, 64], f"wb{l}")
                wt[f"bb{l}"] = load(f"bb{l}", [64, 1], f"bb{l}")
            hw1_t = load("hw1", [64, 3 * 1024], "hw1")
            hb1_t = load("hb1", [P, 8], "hb1")
            hw2_t = load("hw2", [P, 8 * 512], "hw2")
            hb2_t = load("hb2", [P, 4], "hb2")
            hw3_t = load("hw3", [P, 4 * P], "hw3")
            hb3_t = load("hb3", [P, 1], "hb3")

            x0 = sb_p.tile([33, N], F32, tag="x0")
            nc.vector.memset(x0[:], 0.0)
            nc.sync.dma_start(x0[0:6, :], x0cm_d[:])
            nc.vector.memset(x0[32:33, :], 1.0)
            x1 = sb_p.tile([65, N], F32, tag="x1")
            x2 = sb_p.tile([65, N], F32, tag="x2")
            x3 = sb_p.tile([65, N], F32, tag="x3")

            for l, d, xin, xout in ((1, 6, x0, x1), (2, 64, x1, x2),
                                    (3, 64, x2, x3)):
                gsem = [nc.alloc_semaphore(f"gsem{l}"), 0]
                _edge_conv(tc, pools, d, xin, wt[f"wu{l}"], wt[f"wv{l}"],
                           wt[f"ba{l}"], wt[f"wb{l}"], wt[f"bb{l}"], xout,
                           v_hbm[l - 1], ones_t, ident, N, gsem)

            out_sb = sb_p.tile([P, N], F32, tag="out_sb")
            _head(tc, pools, x1, x2, x3, hw1_t, hb1_t, hw2_t, hb2_t,
                  hw3_t, hb3_t, out_sb, N)
            nc.sync.dma_start(out_d[:], out_sb[:])

    nc.compile()
    return nc


def prep_weight_maps(inputs):
    """Host-side reshapes of the (shared) weights into kernel layouts."""
    f = np.asarray
    m = {}
    for l, (wa, ba, wb, bb) in ((1, ("w1a", "b1a", "w1b", "b1b")),
                                (2, ("w2a", "b2a", "w2b", "b2b")),
                                (3, ("w3a", "b3a", "w3b", "b3b"))):
        wa_np = f(inputs[wa], dtype=np.float32)
        d = wa_np.shape[0] // 2
        m[f"wu{l}"] = np.ascontiguousarray(wa_np[:d] - wa_np[d:])
        m[f"wv{l}"] = np.ascontiguousarray(wa_np[d:])
        m[f"ba{l}"] = np.ascontiguousarray(
            np.broadcast_to(f(inputs[ba], dtype=np.float32), (P, 64)))
        wb_np = f(inputs[wb], dtype=np.float32)
        m[f"wb{l}"] = np.ascontiguousarray(np.vstack([wb_np, wb_np]))
        m[f"bb{l}"] = np.ascontiguousarray(
            f(inputs[bb], dtype=np.float32).reshape(64, 1))
    hw1 = f(inputs["hw1"], dtype=np.float32)          # [192, 1024]
    m["hw1"] = np.ascontiguousarray(hw1.reshape(3, 64, 1024).transpose(1, 0, 2)
                                    .reshape(64, 3 * 1024))
    m["hb1"] = np.ascontiguousarray(
        f(inputs["hb1"], dtype=np.float32).reshape(8, P).T)
    hw2 = f(inputs["hw2"], dtype=np.float32)          # [1024, 512]
    m["hw2"] = np.ascontiguousarray(hw2.reshape(8, P, 512).transpose(1, 0, 2)
                                    .reshape(P, 8 * 512))
    m["hb2"] = np.ascontiguousarray(
        f(inputs["hb2"], dtype=np.float32).reshape(4, P).T)
    hw3 = f(inputs["hw3"], dtype=np.float32)          # [512, 128]
    m["hw3"] = np.ascontiguousarray(hw3.reshape(4, P, P).transpose(1, 0, 2)
                                    .reshape(P, 4 * P))
    m["hb3"] = np.ascontiguousarray(
        f(inputs["hb3"], dtype=np.float32).reshape(1, P).T)
    return m


_CACHE = {}


def _get_program(N):
    if N not in _CACHE:
        _CACHE[N] = build_program(N)
    return _CACHE[N]


def kernel(**inputs):
    B = int(inputs["batch_size"])
    x = np.asarray(inputs["x"], dtype=np.float32)
    pos = np.asarray(inputs["pos"], dtype=np.float32)
    N = x.shape[0] // B
    nc = _get_program(N)
    wmap = prep_weight_maps(inputs)

    x0 = np.concatenate([x, pos], axis=-1).reshape(B, N, 6)
    in_maps = []
    for b in range(B):
        im = dict(wmap)
        im["x0cm"] = np.ascontiguousarray(x0[b].T)    # [6, N]
        in_maps.append(im)

    res = run_bass_kernel_spmd(nc, in_maps, core_ids=list(range(B)))
    global LAST_RESULTS
    LAST_RESULTS = res
    out = np.stack([np.ascontiguousarray(r["out_cm"].T) for r in res.results])
    return out.astype(np.float32)


LAST_RESULTS = None


if __name__ == "__main__":
    import reference  # noqa: only for ad-hoc local testing
    ins = reference.setup_inputs()
    o = kernel(**{k: np.asarray(v) if hasattr(v, "shape") else v
                  for k, v in ins.items()})
    print(o.shape, o.dtype)


# revision 13
# speedup vs baseline: 1.0383x; 1.0383x over previous
"""DGCNN (3x DynamicEdgeConv + head MLP) Trainium2 Bass kernel.

Data-parallel over B=8 graphs: one NeuronCore per graph (N=2048 points,
K=30 neighbors). Per core:
  - kNN: negkey[i,j] = 2*f_i.f_j - |f_j|^2 via one augmented PE matmul
    ([f;1]^T @ [2f;-d2]); exact top-30 per row via DVE max8/max_index/
    match_replace rounds (ranking by negkey == ranking by -dist).
  - EdgeConv MLP decomposition: relu(e@wa+ba) = relu(u_i + v_j) with
    u = x@(wa_top-wa_bot)+ba, v = x@wa_bot.  v lives in HBM row-major;
    neighbors fetched with indirect DMA (CCE-add fuses the +u_i).
  - Edge features transposed to channel-major in k-slot pairs on PE,
    relu on ACT (psum->sbuf), second MLP layer (@wb) on PE, max over
    K on DVE strided-view reduce, +bb folded into one ACT pass.
  - Head MLP computed channel-major (output transpose is free on host).
All compute fp32.
"""

import sys

sys.path.insert(0, "/opt/trn_rl_repo")

import os

import numpy as np

import concourse.bass as bass
import concourse.bacc as bacc
import concourse.mybir as mybir
import concourse.tile as tile
from concourse.bass_utils import run_bass_kernel_spmd
from concourse.masks import make_identity

F32 = mybir.dt.float32
U32 = mybir.dt.uint32
U16 = mybir.dt.uint16
I16 = mybir.dt.int16
F16 = mybir.dt.float16
GM = os.environ.get("GM", "perk")
AF = mybir.ActivationFunctionType
ALU = mybir.AluOpType

P = 128
K = 30
KP = 32           # padded neighbor count (pad slots duplicate slot 0)
NEG_BIG = -3.0e38


def _bc_free(ap, count, inner):
    """Insert a 0-stride broadcast dim: [P, inner] -> [P, count, inner]."""
    return bass.AP(ap.tensor, ap.offset, [list(ap.ap[0]), [0, count], [1, inner]])


def _edge_conv(tc, pools, d, x_cm, wu_t, wv_t, ba_t, wb_t, bb_t, x_next,
               v_hbm, ones_t, ident, ident16, N, gsem):
    """One DynamicEdgeConv. x_cm: [d+1, N] sbuf tile (row d == 1.0).
    Writes x_next[0:64, :] (and caller-visible row 64 must be set by caller).
    """
    nc = tc.nc
    NT = N // P
    sb_work, sb_small, ps_s = pools["work"], pools["small"], pools["ps_s"]
    ps_t_pool, ps_h2_pool = pools["ps_t"], pools["ps_h2"]
    da = 32 if d <= 32 else 64   # partition row holding the aug (1 / -d2) entry

    # ---- phase A: u, v, d2, aug_r ----
    sq = sb_work.tile([64, N], F32, tag="sq", bufs=1)
    nc.scalar.activation(sq[0:d, :], x_cm[0:d, :], AF.Square)
    augr = sb_work.tile([65, N], F32, tag="augr", bufs=1)
    if d < da:
        nc.vector.memset(augr[0:da + 1, :], 0.0)
    # rows 0:d = 2*x
    nc.scalar.activation(augr[0:d, :], x_cm[0:d, :], AF.Copy, scale=2.0)
    jblk = min(512, N)
    for jc in range(N // jblk):
        ps_d2 = ps_s.tile([1, jblk], F32, tag="ps64")
        nc.tensor.matmul(ps_d2[:], ones_t[0:d, :], sq[0:d, bass.ts(jc, jblk)],
                         start=True, stop=True)
        # row da = -d2
        nc.scalar.activation(augr[da:da + 1, bass.ts(jc, jblk)], ps_d2[:],
                             AF.Copy, scale=-1.0)

    u_rm = sb_work.tile([P, NT * 64], F32, tag="u_rm", bufs=1)
    v_rm = sb_work.tile([P, NT * 64], F32, tag="v_rm", bufs=1)
    if GM == "dg":
        w_sb = sb_work.tile([P, NT * 256], I16, tag="w_sb", bufs=1, name="w_sb")
    else:
        w_sb = None
    for it in range(NT):
        ps_u = ps_s.tile([P, 64], F32, tag="ps64")
        nc.tensor.matmul(ps_u[:], x_cm[0:d, bass.ts(it, P)], wu_t[0:d, :],
                         start=True, stop=True)
        nc.vector.tensor_add(u_rm[:, bass.ts(it, 64)], ps_u[:], ba_t[:])
        ps_v = ps_s.tile([P, 64], F32, tag="ps64")
        nc.tensor.matmul(ps_v[:], x_cm[0:d, bass.ts(it, P)], wv_t[0:d, :],
                         start=True, stop=True)
        nc.scalar.activation(v_rm[:, bass.ts(it, 64)], ps_v[:], AF.Copy)
    # v -> HBM (row-major [N, 64]) as the gather source
    v_hbm_view = v_hbm[:].rearrange("(t p) c -> p t c", p=P)
    nc.sync.dma_start(v_hbm_view, v_rm[:].rearrange("p (t c) -> p t c", c=64))

    # ---- phase B: per i-tile ----
    for it in range(NT):
        # dist keys: negkey[i, j] = 2 f_i.f_j - d2_j
        negkey = sb_work.tile([P, N], F32, tag="negkey")
        for jc in range(N // jblk):
            ps_nk = ps_s.tile([P, jblk], F32, tag="ps512")
            nc.tensor.matmul(ps_nk[:], x_cm[0:da + 1, bass.ts(it, P)],
                             augr[0:da + 1, bass.ts(jc, jblk)],
                             start=True, stop=True)
            nc.scalar.activation(negkey[:, bass.ts(jc, jblk)], ps_nk[:], AF.Copy)

        # top-30 (+2 dup pads) via 4 rounds of max8
        idx_t = sb_small.tile([P, KP], U16 if GM == "dg" else U32, tag="idx")
        for r in range(4):
            m8 = sb_small.tile([P, 8], F32, tag="m8")
            nc.vector.max(m8[:], negkey[:])
            nc.vector.max_index(idx_t[:, r * 8:(r + 1) * 8], m8[:], negkey[:])
            if r < 3:
                nc.vector.match_replace(negkey[:], m8[:], negkey[:], NEG_BIG)
        # pad slots 30,31 <- slot 0 (self), so max-aggregation is unaffected
        nc.vector.tensor_copy(idx_t[:, 30:31], idx_t[:, 0:1])
        nc.vector.tensor_copy(idx_t[:, 31:32], idx_t[:, 0:1])

        # edge pre-activations: etile[i, k, c] = u_i[c] (+ v_j[c] via CCE add)
        etile = sb_work.tile([P, KP * 64], F32, tag="etile")
        u_slice = u_rm[:, bass.ts(it, 64)]
        if GM != "dg":
            nc.scalar.activation(etile[:], _bc_free(u_slice, KP, 64), AF.Copy)
        ev = etile[:].rearrange("p (k c) -> p k c", c=64)
        if GM == "dg":
            # build wrapped int16 index layout w[e%16, e//16] for e = k*128+i:
            # w[i%16, 8k+i//16] = idx_t[i, k], via PE transposes.
            tps = ps_s.tile([32, P], F16, tag="ps512", name="tps")
            nc.tensor.transpose(tps[:], idx_t[:].bitcast(F16), ident16[:])
            tsb = sb_small.tile([32, P], U16, tag="tsb")
            nc.vector.tensor_copy(tsb[:].bitcast(F16), tps[:])
            for q in range(8):
                tq = ps_s.tile([16, 32], F16, tag="ps512", name="tq")
                nc.tensor.transpose(tq[:], tsb[:, q * 16:(q + 1) * 16].bitcast(F16),
                                    ident16[0:32, 0:32])
                wdst = bass.AP(w_sb.tensor, w_sb[:].offset + it * 256 + q,
                               [[w_sb[:].ap[0][0], 16], [8, 32]])
                nc.vector.tensor_copy(wdst.bitcast(F16), tq[:])
            # replicate wrapped idxs to all 8 Q7 core groups (16 -> 128 rows)
            for lo, n in ((16, 16), (32, 32), (64, 64)):
                nc.sync.dma_start(w_sb[lo:lo + n, bass.ts(it, 256)],
                                  w_sb[0:n, bass.ts(it, 256)])
            with tc.tile_critical():
                nc.gpsimd.dma_gather(
                    out_ap=ev, in_ap=v_hbm[:],
                    idxs_ap=w_sb[:, bass.ts(it, 256)],
                    num_idxs=P * KP, num_idxs_reg=P * KP,
                    elem_size=64, queue_num=0).then_inc(gsem[0], 16)
                gsem[1] += 16
                nc.gpsimd.wait_ge(gsem[0], gsem[1])
            nc.vector.tensor_add(etile[:], etile[:],
                                 _bc_free(u_slice, KP, 64))
        else:
            with tc.tile_critical():
                for k in range(KP):
                    nc.gpsimd.indirect_dma_start(
                        out=ev[:, k, :], out_offset=None,
                        in_=v_hbm[:],
                        in_offset=bass.IndirectOffsetOnAxis(
                            ap=idx_t[:, k:k + 1], axis=0),
                        compute_op=ALU.add).then_inc(gsem[0], 16)
                gsem[1] += 16 * KP
                nc.gpsimd.wait_ge(gsem[0], gsem[1])

        # k-pair transpose -> relu -> @wb -> segment-max
        acc = None
        for g in range(KP // 8):
            ps_t = ps_t_pool.tile([P, 512], F32, tag="ps_t")
            for t in range(4):
                tt = g * 4 + t
                nc.tensor.transpose(ps_t[:, bass.ts(t, P)],
                                    etile[:, bass.ts(tt, P)], ident[:])
            rhs = sb_small.tile([P, 512], F32, tag="rhs")
            nc.scalar.activation(rhs[:], ps_t[:], AF.Relu)
            hp = ps_h2_pool.tile([64, 1024], F32, tag="ps_h2")
            nc.tensor.matmul(hp[:, 0:512], wb_t[0:64, :], rhs[0:64, :],
                             start=True, stop=True)
            nc.tensor.matmul(hp[:, 512:1024], wb_t[64:128, :], rhs[64:128, :],
                             start=True, stop=True)
            # view [64, 128 i, 2 parity, 4 t] -> reduce XY -> [64, 128]
            hview = bass.AP(hp.tensor, 0,
                            [[hp[:].ap[0][0], 64], [1, P], [512, 2], [P, 4]])
            if g == 0:
                acc = sb_small.tile([64, P], F32, tag="seg_acc")
                nc.vector.tensor_reduce(acc[:], hview, mybir.AxisListType.XY,
                                        ALU.max)
            else:
                tmp = sb_small.tile([64, P], F32, tag="seg_tmp")
                nc.vector.tensor_reduce(tmp[:], hview, mybir.AxisListType.XY,
                                        ALU.max)
                dst = (x_next[0:64, bass.ts(it, P)] if g == KP // 8 - 1
                       else acc[:])
                nc.vector.tensor_tensor(dst, acc[:], tmp[:], ALU.max)

    # +bb (per-channel bias) in place
    nc.scalar.activation(x_next[0:64, :], x_next[0:64, :], AF.Identity,
                         bias=bb_t[:, 0:1])
    # aug ones row for the next conv's lhsT
    nc.vector.memset(x_next[64:65, :], 1.0)


def _head(tc, pools, x1, x2, x3, hw1_t, hb1_t, hw2_t, hb2_t, hw3_t, hb3_t,
          out_sb, N):
    """Head MLP, channel-major: relu(feat@hw1+hb1) -> relu(@hw2+hb2) -> @hw3+hb3."""
    nc = tc.nc
    sb_head, ps_s = pools["head"], pools["ps_s"]
    xs = [x1, x2, x3]
    blk = min(512, N)
    for ib in range(N // blk):
        isl = bass.ts(ib, blk)
        h1 = sb_head.tile([P, 8 * blk], F32, tag="h1")
        for c in range(8):
            ps = ps_s.tile([P, blk], F32, tag="ps512")
            for kc in range(3):
                nc.tensor.matmul(ps[:],
                                 hw1_t[:, kc * 1024 + c * P:kc * 1024 + (c + 1) * P],
                                 xs[kc][0:64, isl],
                                 start=(kc == 0), stop=(kc == 2))
            nc.scalar.activation(h1[:, bass.ts(c, blk)], ps[:], AF.Relu,
                                 bias=hb1_t[:, c:c + 1])
        h2 = sb_head.tile([P, 4 * blk], F32, tag="h2")
        for c in range(4):
            ps = ps_s.tile([P, blk], F32, tag="ps512")
            for kc in range(8):
                nc.tensor.matmul(ps[:],
                                 hw2_t[:, kc * 512 + c * P:kc * 512 + (c + 1) * P],
                                 h1[:, bass.ts(kc, blk)],
                                 start=(kc == 0), stop=(kc == 7))
            nc.scalar.activation(h2[:, bass.ts(c, blk)], ps[:], AF.Relu,
                                 bias=hb2_t[:, c:c + 1])
        ps = ps_s.tile([P, blk], F32, tag="ps512")
        for kc in range(4):
            nc.tensor.matmul(ps[:], hw3_t[:, bass.ts(kc, P)],
                             h2[:, bass.ts(kc, blk)],
                             start=(kc == 0), stop=(kc == 3))
        nc.scalar.activation(out_sb[:, isl], ps[:], AF.Identity,
                             bias=hb3_t[:, 0:1])


def build_program(N):
    nc = bacc.Bacc("TRN2", target_bir_lowering=False, debug=False)
    NT = N // P

    x0cm_d = nc.dram_tensor("x0cm", [6, N], F32, kind="ExternalInput")
    w_d = {}
    for l, d in ((1, 6), (2, 64), (3, 64)):
        w_d[f"wu{l}"] = nc.dram_tensor(f"wu{l}", [d, 64], F32, kind="ExternalInput")
        w_d[f"wv{l}"] = nc.dram_tensor(f"wv{l}", [d, 64], F32, kind="ExternalInput")
        w_d[f"ba{l}"] = nc.dram_tensor(f"ba{l}", [P, 64], F32, kind="ExternalInput")
        w_d[f"wb{l}"] = nc.dram_tensor(f"wb{l}", [P, 64], F32, kind="ExternalInput")
        w_d[f"bb{l}"] = nc.dram_tensor(f"bb{l}", [64, 1], F32, kind="ExternalInput")
    w_d["hw1"] = nc.dram_tensor("hw1", [64, 3 * 1024], F32, kind="ExternalInput")
    w_d["hb1"] = nc.dram_tensor("hb1", [P, 8], F32, kind="ExternalInput")
    w_d["hw2"] = nc.dram_tensor("hw2", [P, 8 * 512], F32, kind="ExternalInput")
    w_d["hb2"] = nc.dram_tensor("hb2", [P, 4], F32, kind="ExternalInput")
    w_d["hw3"] = nc.dram_tensor("hw3", [P, 4 * P], F32, kind="ExternalInput")
    w_d["hb3"] = nc.dram_tensor("hb3", [P, 1], F32, kind="ExternalInput")
    out_d = nc.dram_tensor("out_cm", [P, N], F32, kind="ExternalOutput")
    v_hbm = [nc.dram_tensor(f"vscratch{l}", [N, 64], F32) for l in (1, 2, 3)]

    with tile.TileContext(nc) as tc:
        with tc.tile_pool(name="persist", bufs=1) as sb_p, \
             tc.tile_pool(name="work", bufs=2) as sb_work, \
             tc.tile_pool(name="small", bufs=3) as sb_small, \
             tc.tile_pool(name="head", bufs=1) as sb_head, \
             tc.tile_pool(name="ps_s", bufs=2, space="PSUM") as ps_s, \
             tc.tile_pool(name="ps_t", bufs=2, space="PSUM") as ps_t_pool, \
             tc.tile_pool(name="ps_h2", bufs=1, space="PSUM") as ps_h2_pool:
            pools = {"work": sb_work, "small": sb_small, "head": sb_head,
                     "ps_s": ps_s, "ps_t": ps_t_pool, "ps_h2": ps_h2_pool}

            ident = sb_p.tile([P, P], F32, tag="ident")
            make_identity(nc, ident[:])
            ident16 = sb_p.tile([P, P], F16, tag="ident16")
            make_identity(nc, ident16[:])
            ones_t = sb_p.tile([64, 1], F32, tag="ones")
            nc.vector.memset(ones_t[:], 1.0)

            def load(name, shape, tag):
                t = sb_p.tile(shape, F32, tag=tag)
                nc.sync.dma_start(t[:], w_d[name][:])
                return t

            wt = {}
            for l, d in ((1, 6), (2, 64), (3, 64)):
                wt[f"wu{l}"] = load(f"wu{l}", [d, 64], f"wu{l}")
                wt[f"wv{l}"] = load(f"wv{l}", [d, 64], f"wv{l}")
                wt[f"ba{l}"] = load(f"ba{l}", [P, 64], f"ba{l}")
                wt[f"wb{l}"] = load(f"wb{l}", [P, 64], f"wb{l}")
                wt[f"bb{l}"] = load(f"bb{l}", [64, 1], f"bb{l}")
            hw1_t = load("hw1", [64, 3 * 1024], "hw1")
            hb1_t = load("hb1", [P, 8], "hb1")
            hw2_t = load("hw2", [P, 8 * 512], "hw2")
            hb2_t = load("hb2", [P, 4], "hb2")
            hw3_t = load("hw3", [P, 4 * P], "hw3")
            hb3_t = load("hb3", [P, 1], "hb3")

            x0 = sb_p.tile([33, N], F32, tag="x0")
            nc.vector.memset(x0[:], 0.0)
            nc.sync.dma_start(x0[0:6, :], x0cm_d[:])
            nc.vector.memset(x0[32:33, :], 1.0)
            x1 = sb_p.tile([65, N], F32, tag="x1")
            x2 = sb_p.tile([65, N], F32, tag="x2")
            x3 = sb_p.tile([65, N], F32, tag="x3")

            for l, d, xin, xout in ((1, 6, x0, x1), (2, 64, x1, x2),
                                    (3, 64, x2, x3)):
                gsem = [nc.alloc_semaphore(f"gsem{l}"), 0]
                _edge_conv(tc, pools, d, xin, wt[f"wu{l}"], wt[f"wv{l}"],
                           wt[f"ba{l}"], wt[f"wb{l}"], wt[f"bb{l}"], xout,
                           v_hbm[l - 1], ones_t, ident, ident16, N, gsem)

            out_sb = sb_p.tile([P, N], F32, tag="out_sb")
            _head(tc, pools, x1, x2, x3, hw1_t, hb1_t, hw2_t, hb2_t,
                  hw3_t, hb3_t, out_sb, N)
            nc.sync.dma_start(out_d[:], out_sb[:])

    nc.compile()
    return nc


def prep_weight_maps(inputs):
    """Host-side reshapes of the (shared) weights into kernel layouts."""
    f = np.asarray
    m = {}
    for l, (wa, ba, wb, bb) in ((1, ("w1a", "b1a", "w1b", "b1b")),
                                (2, ("w2a", "b2a", "w2b", "b2b")),
                                (3, ("w3a", "b3a", "w3b", "b3b"))):
        wa_np = f(inputs[wa], dtype=np.float32)
        d = wa_np.shape[0] // 2
        m[f"wu{l}"] = np.ascontiguousarray(wa_np[:d] - wa_np[d:])
        m[f"wv{l}"] = np.ascontiguousarray(wa_np[d:])
        m[f"ba{l}"] = np.ascontiguousarray(
            np.broadcast_to(f(inputs[ba], dtype=np.float32), (P, 64)))
        wb_np = f(inputs[wb], dtype=np.float32)
        m[f"wb{l}"] = np.ascontiguousarray(np.vstack([wb_np, wb_np]))
        m[f"bb{l}"] = np.ascontiguousarray(
            f(inputs[bb], dtype=np.float32).reshape(64, 1))
    hw1 = f(inputs["hw1"], dtype=np.float32)          # [192, 1024]
    m["hw1"] = np.ascontiguousarray(hw1.reshape(3, 64, 1024).transpose(1, 0, 2)
                                    .reshape(64, 3 * 1024))
    m["hb1"] = np.ascontiguousarray(
        f(inputs["hb1"], dtype=np.float32).reshape(8, P).T)
    hw2 = f(inputs["hw2"], dtype=np.float32)          # [1024, 512]
    m["hw2"] = np.ascontiguousarray(hw2.reshape(8, P, 512).transpose(1, 0, 2)
                                    .reshape(P, 8 * 512))
    m["hb2"] = np.ascontiguousarray(
        f(inputs["hb2"], dtype=np.float32).reshape(4, P).T)
    hw3 = f(inputs["hw3"], dtype=np.float32)          # [512, 128]
    m["hw3"] = np.ascontiguousarray(hw3.reshape(4, P, P).transpose(1, 0, 2)
                                    .reshape(P, 4 * P))
    m["hb3"] = np.ascontiguousarray(
        f(inputs["hb3"], dtype=np.float32).reshape(1, P).T)
    return m


_CACHE = {}


def _get_program(N):
    if N not in _CACHE:
        _CACHE[N] = build_program(N)
    return _CACHE[N]


def kernel(**inputs):
    B = int(inputs["batch_size"])
    x = np.asarray(inputs["x"], dtype=np.float32)
    pos = np.asarray(inputs["pos"], dtype=np.float32)
    N = x.shape[0] // B
    nc = _get_program(N)
    wmap = prep_weight_maps(inputs)

    x0 = np.concatenate([x, pos], axis=-1).reshape(B, N, 6)
    in_maps = []
    for b in range(B):
        im = dict(wmap)
        im["x0cm"] = np.ascontiguousarray(x0[b].T)    # [6, N]
        in_maps.append(im)

    res = run_bass_kernel_spmd(nc, in_maps, core_ids=list(range(B)))
    global LAST_RESULTS
    LAST_RESULTS = res
    out = np.stack([np.ascontiguousarray(r["out_cm"].T) for r in res.results])
    return out.astype(np.float32)


LAST_RESULTS = None


if __name__ == "__main__":
    import reference  # noqa: only for ad-hoc local testing
    ins = reference.setup_inputs()
    o = kernel(**{k: np.asarray(v) if hasattr(v, "shape") else v
                  for k, v in ins.items()})
    print(o.shape, o.dtype)


# revision 17
# speedup vs baseline: 49.1559x; 47.3415x over previous
"""DGCNN (3x DynamicEdgeConv + head MLP) Trainium2 Bass kernel.

Data-parallel over B=8 graphs: one NeuronCore per graph (N=2048 points,
K=30 neighbors). Per core:
  - kNN: negkey[i,j] = 2*f_i.f_j - |f_j|^2 via one augmented PE matmul
    ([f;1]^T @ [2f;-d2]); exact top-30 per row via DVE max8/max_index/
    match_replace rounds (ranking by negkey == ranking by -dist).
  - EdgeConv MLP decomposition: relu(e@wa+ba) = relu(u_i + v_j) with
    u = x@(wa_top-wa_bot)+ba, v = x@wa_bot.  v lives in HBM row-major;
    neighbors fetched with indirect DMA (CCE-add fuses the +u_i).
  - Edge features transposed to channel-major in k-slot pairs on PE,
    relu on ACT (psum->sbuf), second MLP layer (@wb) on PE, max over
    K on DVE strided-view reduce, +bb folded into one ACT pass.
  - Head MLP computed channel-major (output transpose is free on host).
All compute fp32.
"""

import sys

sys.path.insert(0, "/opt/trn_rl_repo")

import os

import numpy as np

import concourse.bass as bass
import concourse.bacc as bacc
import concourse.mybir as mybir
import concourse.tile as tile
from concourse.bass_utils import run_bass_kernel_spmd
from concourse.masks import make_identity
from concourse import library_config

F32 = mybir.dt.float32
U32 = mybir.dt.uint32
U16 = mybir.dt.uint16
I16 = mybir.dt.int16
F16 = mybir.dt.float16
GM = os.environ.get("GM", "perk")
AF = mybir.ActivationFunctionType
ALU = mybir.AluOpType

P = 128
K = 30
KP = 32           # padded neighbor count (pad slots duplicate slot 0)
NEG_BIG = -3.0e38


def _bc_free(ap, count, inner):
    """Insert a 0-stride broadcast dim: [P, inner] -> [P, count, inner]."""
    return bass.AP(ap.tensor, ap.offset, [list(ap.ap[0]), [0, count], [1, inner]])


def _edge_conv(tc, pools, d, x_cm, wu_t, wv_t, ba_t, wb_t, bb_t, x_next,
               v_hbm, ones_t, ident, ident16, N, gsem):
    """One DynamicEdgeConv. x_cm: [d+1, N] sbuf tile (row d == 1.0).
    Writes x_next[0:64, :] (and caller-visible row 64 must be set by caller).
    """
    nc = tc.nc
    NT = N // P
    sb_work, sb_small, ps_s = pools["work"], pools["small"], pools["ps_s"]
    ps_t_pool, ps_h2_pool = pools["ps_t"], pools["ps_h2"]
    da = 32 if d <= 32 else 64   # partition row holding the aug (1 / -d2) entry

    # ---- phase A: u, v, d2, aug_r ----
    sq = sb_work.tile([64, N], F32, tag="sq", bufs=1)
    nc.scalar.activation(sq[0:d, :], x_cm[0:d, :], AF.Square)
    augr = sb_work.tile([65, N], F32, tag="augr", bufs=1)
    if d < da:
        nc.vector.memset(augr[0:da + 1, :], 0.0)
    # rows 0:d = 2*x
    nc.scalar.activation(augr[0:d, :], x_cm[0:d, :], AF.Copy, scale=2.0)
    jblk = min(512, N)
    for jc in range(N // jblk):
        ps_d2 = ps_s.tile([1, jblk], F32, tag="ps64")
        nc.tensor.matmul(ps_d2[:], ones_t[0:d, :], sq[0:d, bass.ts(jc, jblk)],
                         start=True, stop=True)
        # row da = -d2
        nc.scalar.activation(augr[da:da + 1, bass.ts(jc, jblk)], ps_d2[:],
                             AF.Copy, scale=-1.0)

    u_rm = sb_work.tile([P, NT * 64], F32, tag="u_rm", bufs=1)
    v_rm = sb_work.tile([P, NT * 64], F32, tag="v_rm", bufs=1)
    if GM == "dg":
        w_sb = sb_work.tile([P, NT * 256], I16, tag="w_sb", bufs=1, name="w_sb")
    else:
        w_sb = None
    for it in range(NT):
        ps_u = ps_s.tile([P, 64], F32, tag="ps64")
        nc.tensor.matmul(ps_u[:], x_cm[0:d, bass.ts(it, P)], wu_t[0:d, :],
                         start=True, stop=True)
        nc.vector.tensor_add(u_rm[:, bass.ts(it, 64)], ps_u[:], ba_t[:])
        ps_v = ps_s.tile([P, 64], F32, tag="ps64")
        nc.tensor.matmul(ps_v[:], x_cm[0:d, bass.ts(it, P)], wv_t[0:d, :],
                         start=True, stop=True)
        nc.scalar.activation(v_rm[:, bass.ts(it, 64)], ps_v[:], AF.Copy)
    # v -> HBM (row-major [N, 64]) as the gather source
    v_hbm_view = v_hbm[:].rearrange("(t p) c -> p t c", p=P)
    nc.sync.dma_start(v_hbm_view, v_rm[:].rearrange("p (t c) -> p t c", c=64))

    # ---- phase B: per i-tile ----
    for it in range(NT):
        # dist keys: negkey[i, j] = 2 f_i.f_j - d2_j
        negkey = sb_work.tile([P, N], F32, tag="negkey")
        for jc in range(N // jblk):
            ps_nk = ps_s.tile([P, jblk], F32, tag="ps512")
            nc.tensor.matmul(ps_nk[:], x_cm[0:da + 1, bass.ts(it, P)],
                             augr[0:da + 1, bass.ts(jc, jblk)],
                             start=True, stop=True)
            nc.scalar.activation(negkey[:, bass.ts(jc, jblk)], ps_nk[:], AF.Copy)

        # top-30 (+2 dup pads) via 4 rounds of max8
        idx_t = sb_small.tile([P, KP], U16 if GM == "dg" else U32, tag="idx")
        for r in range(4):
            m8 = sb_small.tile([P, 8], F32, tag="m8")
            nc.vector.max(m8[:], negkey[:])
            nc.vector.max_index(idx_t[:, r * 8:(r + 1) * 8], m8[:], negkey[:])
            if r < 3:
                nc.vector.match_replace(negkey[:], m8[:], negkey[:], NEG_BIG)
        # pad slots 30,31 <- slot 0 (self), so max-aggregation is unaffected
        nc.vector.tensor_copy(idx_t[:, 30:31], idx_t[:, 0:1])
        nc.vector.tensor_copy(idx_t[:, 31:32], idx_t[:, 0:1])

        # edge pre-activations: etile[i, k, c] = u_i[c] (+ v_j[c] via CCE add)
        etile = sb_work.tile([P, KP * 64], F32, tag="etile")
        u_slice = u_rm[:, bass.ts(it, 64)]
        if GM != "dg":
            nc.scalar.activation(etile[:], _bc_free(u_slice, KP, 64), AF.Copy)
        ev = etile[:].rearrange("p (k c) -> p k c", c=64)
        if GM == "dg":
            # build wrapped int16 index layout w[e%16, e//16] for e = k*128+i:
            # w[i%16, 8k+i//16] = idx_t[i, k], via PE transposes.
            tps = ps_s.tile([32, P], F16, tag="ps512", name="tps")
            nc.tensor.transpose(tps[:], idx_t[:].bitcast(F16), ident16[:])
            tsb = sb_small.tile([32, P], U16, tag="tsb")
            nc.vector.tensor_copy(tsb[:].bitcast(F16), tps[:])
            for q in range(8):
                tq = ps_s.tile([16, 32], F16, tag="ps512", name="tq")
                nc.tensor.transpose(tq[:], tsb[:, q * 16:(q + 1) * 16].bitcast(F16),
                                    ident16[0:32, 0:32])
                wdst = bass.AP(w_sb.tensor, w_sb[:].offset + it * 256 + q,
                               [[w_sb[:].ap[0][0], 16], [8, 32]])
                nc.vector.tensor_copy(wdst.bitcast(F16), tq[:])
            # replicate wrapped idxs to all 8 Q7 core groups (16 -> 128 rows)
            for lo, n in ((16, 16), (32, 32), (64, 64)):
                nc.sync.dma_start(w_sb[lo:lo + n, bass.ts(it, 256)],
                                  w_sb[0:n, bass.ts(it, 256)])
            with tc.tile_critical():
                nc.gpsimd.dma_gather(
                    out_ap=ev, in_ap=v_hbm[:],
                    idxs_ap=w_sb[:, bass.ts(it, 256)],
                    num_idxs=P * KP, num_idxs_reg=P * KP,
                    elem_size=64, queue_num=0).then_inc(gsem[0], 16)
                gsem[1] += 16
                nc.gpsimd.wait_ge(gsem[0], gsem[1])
            nc.vector.tensor_add(etile[:], etile[:],
                                 _bc_free(u_slice, KP, 64))
        else:
            with tc.tile_critical():
                for k in range(KP):
                    nc.gpsimd.indirect_dma_start(
                        out=ev[:, k, :], out_offset=None,
                        in_=v_hbm[:],
                        in_offset=bass.IndirectOffsetOnAxis(
                            ap=idx_t[:, k:k + 1], axis=0),
                        compute_op=ALU.add).then_inc(gsem[0], 16)
                gsem[1] += 16 * KP
                nc.gpsimd.wait_ge(gsem[0], gsem[1])

        # k-pair transpose -> relu -> @wb -> segment-max
        acc = None
        for g in range(KP // 8):
            ps_t = ps_t_pool.tile([P, 512], F32, tag="ps_t")
            for t in range(4):
                tt = g * 4 + t
                nc.tensor.transpose(ps_t[:, bass.ts(t, P)],
                                    etile[:, bass.ts(tt, P)], ident[:])
            rhs = sb_small.tile([P, 512], F32, tag="rhs")
            nc.scalar.activation(rhs[:], ps_t[:], AF.Relu)
            hp = ps_h2_pool.tile([64, 1024], F32, tag="ps_h2")
            nc.tensor.matmul(hp[:, 0:512], wb_t[0:64, :], rhs[0:64, :],
                             start=True, stop=True)
            nc.tensor.matmul(hp[:, 512:1024], wb_t[64:128, :], rhs[64:128, :],
                             start=True, stop=True)
            # view [64, 128 i, 2 parity, 4 t] -> reduce XY -> [64, 128]
            hview = bass.AP(hp.tensor, 0,
                            [[hp[:].ap[0][0], 64], [1, P], [512, 2], [P, 4]])
            if g == 0:
                acc = sb_small.tile([64, P], F32, tag="seg_acc")
                nc.vector.tensor_reduce(acc[:], hview, mybir.AxisListType.XY,
                                        ALU.max)
            else:
                tmp = sb_small.tile([64, P], F32, tag="seg_tmp")
                nc.vector.tensor_reduce(tmp[:], hview, mybir.AxisListType.XY,
                                        ALU.max)
                dst = (x_next[0:64, bass.ts(it, P)] if g == KP // 8 - 1
                       else acc[:])
                nc.vector.tensor_tensor(dst, acc[:], tmp[:], ALU.max)

    # +bb (per-channel bias) in place
    nc.scalar.activation(x_next[0:64, :], x_next[0:64, :], AF.Identity,
                         bias=bb_t[:, 0:1])
    # aug ones row for the next conv's lhsT
    nc.vector.memset(x_next[64:65, :], 1.0)


def _head(tc, pools, x1, x2, x3, hw1_t, hb1_t, hw2_t, hb2_t, hw3_t, hb3_t,
          out_sb, N):
    """Head MLP, channel-major: relu(feat@hw1+hb1) -> relu(@hw2+hb2) -> @hw3+hb3."""
    nc = tc.nc
    sb_head, ps_s = pools["head"], pools["ps_s"]
    xs = [x1, x2, x3]
    blk = min(512, N)
    for ib in range(N // blk):
        isl = bass.ts(ib, blk)
        h1 = sb_head.tile([P, 8 * blk], F32, tag="h1")
        for c in range(8):
            ps = ps_s.tile([P, blk], F32, tag="ps512")
            for kc in range(3):
                nc.tensor.matmul(ps[:],
                                 hw1_t[:, kc * 1024 + c * P:kc * 1024 + (c + 1) * P],
                                 xs[kc][0:64, isl],
                                 start=(kc == 0), stop=(kc == 2))
            nc.scalar.activation(h1[:, bass.ts(c, blk)], ps[:], AF.Relu,
                                 bias=hb1_t[:, c:c + 1])
        h2 = sb_head.tile([P, 4 * blk], F32, tag="h2")
        for c in range(4):
            ps = ps_s.tile([P, blk], F32, tag="ps512")
            for kc in range(8):
                nc.tensor.matmul(ps[:],
                                 hw2_t[:, kc * 512 + c * P:kc * 512 + (c + 1) * P],
                                 h1[:, bass.ts(kc, blk)],
                                 start=(kc == 0), stop=(kc == 7))
            nc.scalar.activation(h2[:, bass.ts(c, blk)], ps[:], AF.Relu,
                                 bias=hb2_t[:, c:c + 1])
        ps = ps_s.tile([P, blk], F32, tag="ps512")
        for kc in range(4):
            nc.tensor.matmul(ps[:], hw3_t[:, bass.ts(kc, P)],
                             h2[:, bass.ts(kc, blk)],
                             start=(kc == 0), stop=(kc == 3))
        nc.scalar.activation(out_sb[:, isl], ps[:], AF.Identity,
                             bias=hb3_t[:, 0:1])


def build_program(N, repeat=1):
    nc = bacc.Bacc("TRN2", target_bir_lowering=False, debug=False)
    NT = N // P

    x0cm_d = nc.dram_tensor("x0cm", [6, N], F32, kind="ExternalInput")
    w_d = {}
    for l, d in ((1, 6), (2, 64), (3, 64)):
        w_d[f"wu{l}"] = nc.dram_tensor(f"wu{l}", [d, 64], F32, kind="ExternalInput")
        w_d[f"wv{l}"] = nc.dram_tensor(f"wv{l}", [d, 64], F32, kind="ExternalInput")
        w_d[f"ba{l}"] = nc.dram_tensor(f"ba{l}", [P, 64], F32, kind="ExternalInput")
        w_d[f"wb{l}"] = nc.dram_tensor(f"wb{l}", [P, 64], F32, kind="ExternalInput")
        w_d[f"bb{l}"] = nc.dram_tensor(f"bb{l}", [64, 1], F32, kind="ExternalInput")
    w_d["hw1"] = nc.dram_tensor("hw1", [64, 3 * 1024], F32, kind="ExternalInput")
    w_d["hb1"] = nc.dram_tensor("hb1", [P, 8], F32, kind="ExternalInput")
    w_d["hw2"] = nc.dram_tensor("hw2", [P, 8 * 512], F32, kind="ExternalInput")
    w_d["hb2"] = nc.dram_tensor("hb2", [P, 4], F32, kind="ExternalInput")
    w_d["hw3"] = nc.dram_tensor("hw3", [P, 4 * P], F32, kind="ExternalInput")
    w_d["hb3"] = nc.dram_tensor("hb3", [P, 1], F32, kind="ExternalInput")
    out_d = nc.dram_tensor("out_cm", [P, N], F32, kind="ExternalOutput")
    v_hbm = [nc.dram_tensor(f"vscratch{l}", [N, 64], F32) for l in (1, 2, 3)]

    with tile.TileContext(nc) as tc:
        with tc.tile_pool(name="persist", bufs=1) as sb_p, \
             tc.tile_pool(name="work", bufs=2) as sb_work, \
             tc.tile_pool(name="small", bufs=3) as sb_small, \
             tc.tile_pool(name="head", bufs=1) as sb_head, \
             tc.tile_pool(name="ps_s", bufs=2, space="PSUM") as ps_s, \
             tc.tile_pool(name="ps_t", bufs=2, space="PSUM") as ps_t_pool, \
             tc.tile_pool(name="ps_h2", bufs=1, space="PSUM") as ps_h2_pool:
            pools = {"work": sb_work, "small": sb_small, "head": sb_head,
                     "ps_s": ps_s, "ps_t": ps_t_pool, "ps_h2": ps_h2_pool}

            ident = sb_p.tile([P, P], F32, tag="ident")
            make_identity(nc, ident[:])
            ident16 = sb_p.tile([P, P], F16, tag="ident16")
            make_identity(nc, ident16[:])
            if GM == "dg":
                with tc.tile_critical():
                    nc.gpsimd.load_library(library_config.mlp)
            ones_t = sb_p.tile([64, 1], F32, tag="ones")
            nc.vector.memset(ones_t[:], 1.0)

            def load(name, shape, tag):
                t = sb_p.tile(shape, F32, tag=tag)
                nc.sync.dma_start(t[:], w_d[name][:])
                return t

            wt = {}
            for l, d in ((1, 6), (2, 64), (3, 64)):
                wt[f"wu{l}"] = load(f"wu{l}", [d, 64], f"wu{l}")
                wt[f"wv{l}"] = load(f"wv{l}", [d, 64], f"wv{l}")
                wt[f"ba{l}"] = load(f"ba{l}", [P, 64], f"ba{l}")
                wt[f"wb{l}"] = load(f"wb{l}", [P, 64], f"wb{l}")
                wt[f"bb{l}"] = load(f"bb{l}", [64, 1], f"bb{l}")
            hw1_t = load("hw1", [64, 3 * 1024], "hw1")
            hb1_t = load("hb1", [P, 8], "hb1")
            hw2_t = load("hw2", [P, 8 * 512], "hw2")
            hb2_t = load("hb2", [P, 4], "hb2")
            hw3_t = load("hw3", [P, 4 * P], "hw3")
            hb3_t = load("hb3", [P, 1], "hb3")

            x0 = sb_p.tile([33, N], F32, tag="x0")
            nc.vector.memset(x0[:], 0.0)
            nc.sync.dma_start(x0[0:6, :], x0cm_d[:])
            nc.vector.memset(x0[32:33, :], 1.0)
            x1 = sb_p.tile([65, N], F32, tag="x1")
            x2 = sb_p.tile([65, N], F32, tag="x2")
            x3 = sb_p.tile([65, N], F32, tag="x3")

            import contextlib

            gsems = {l: nc.alloc_semaphore(f"gsem{l}") for l in (1, 2, 3)}
            out_sb = sb_p.tile([P, N], F32, tag="out_sb")

            def body():
                for l, d, xin, xout in ((1, 6, x0, x1), (2, 64, x1, x2),
                                        (3, 64, x2, x3)):
                    gsem = [gsems[l], 0]
                    _edge_conv(tc, pools, d, xin, wt[f"wu{l}"], wt[f"wv{l}"],
                               wt[f"ba{l}"], wt[f"wb{l}"], wt[f"bb{l}"], xout,
                               v_hbm[l - 1], ones_t, ident, ident16, N, gsem)
                _head(tc, pools, x1, x2, x3, hw1_t, hb1_t, hw2_t, hb2_t,
                      hw3_t, hb3_t, out_sb, N)
                nc.sync.dma_start(out_d[:], out_sb[:])

            if repeat > 1:
                with tc.For_i(0, repeat, 1):
                    body()
            else:
                body()

    nc.compile()
    return nc


def prep_weight_maps(inputs):
    """Host-side reshapes of the (shared) weights into kernel layouts."""
    f = np.asarray
    m = {}
    for l, (wa, ba, wb, bb) in ((1, ("w1a", "b1a", "w1b", "b1b")),
                                (2, ("w2a", "b2a", "w2b", "b2b")),
                                (3, ("w3a", "b3a", "w3b", "b3b"))):
        wa_np = f(inputs[wa], dtype=np.float32)
        d = wa_np.shape[0] // 2
        m[f"wu{l}"] = np.ascontiguousarray(wa_np[:d] - wa_np[d:])
        m[f"wv{l}"] = np.ascontiguousarray(wa_np[d:])
        m[f"ba{l}"] = np.ascontiguousarray(
            np.broadcast_to(f(inputs[ba], dtype=np.float32), (P, 64)))
        wb_np = f(inputs[wb], dtype=np.float32)
        m[f"wb{l}"] = np.ascontiguousarray(np.vstack([wb_np, wb_np]))
        m[f"bb{l}"] = np.ascontiguousarray(
            f(inputs[bb], dtype=np.float32).reshape(64, 1))
    hw1 = f(inputs["hw1"], dtype=np.float32)          # [192, 1024]
    m["hw1"] = np.ascontiguousarray(hw1.reshape(3, 64, 1024).transpose(1, 0, 2)
                                    .reshape(64, 3 * 1024))
    m["hb1"] = np.ascontiguousarray(
        f(inputs["hb1"], dtype=np.float32).reshape(8, P).T)
    hw2 = f(inputs["hw2"], dtype=np.float32)          # [1024, 512]
    m["hw2"] = np.ascontiguousarray(hw2.reshape(8, P, 512).transpose(1, 0, 2)
                                    .reshape(P, 8 * 512))
    m["hb2"] = np.ascontiguousarray(
        f(inputs["hb2"], dtype=np.float32).reshape(4, P).T)
    hw3 = f(inputs["hw3"], dtype=np.float32)          # [512, 128]
    m["hw3"] = np.ascontiguousarray(hw3.reshape(4, P, P).transpose(1, 0, 2)
                                    .reshape(P, 4 * P))
    m["hb3"] = np.ascontiguousarray(
        f(inputs["hb3"], dtype=np.float32).reshape(1, P).T)
    return m


_CACHE = {}


def _get_program(N):
    repeat = int(os.environ.get("REPEAT", "1"))
    key = (N, repeat)
    if key not in _CACHE:
        _CACHE[key] = build_program(N, repeat)
    return _CACHE[key]


def kernel(**inputs):
    B = int(inputs["batch_size"])
    x = np.asarray(inputs["x"], dtype=np.float32)
    pos = np.asarray(inputs["pos"], dtype=np.float32)
    N = x.shape[0] // B
    nc = _get_program(N)
    wmap = prep_weight_maps(inputs)

    x0 = np.concatenate([x, pos], axis=-1).reshape(B, N, 6)
    in_maps = []
    for b in range(B):
        im = dict(wmap)
        im["x0cm"] = np.ascontiguousarray(x0[b].T)    # [6, N]
        in_maps.append(im)

    res = run_bass_kernel_spmd(nc, in_maps, core_ids=list(range(B)))
    global LAST_RESULTS
    LAST_RESULTS = res
    out = np.stack([np.ascontiguousarray(r["out_cm"].T) for r in res.results])
    return out.astype(np.float32)


LAST_RESULTS = None


if __name__ == "__main__":
    import reference  # noqa: only for ad-hoc local testing
    ins = reference.setup_inputs()
    o = kernel(**{k: np.asarray(v) if hasattr(v, "shape") else v
                  for k, v in ins.items()})
    print(o.shape, o.dtype)


# revision 19
# speedup vs baseline: 140.3474x; 2.8551x over previous
"""DGCNN (3x DynamicEdgeConv + head MLP) Trainium2 Bass kernel.

Data-parallel over B=8 graphs: one NeuronCore per graph (N=2048 points,
K=30 neighbors). Per core:
  - kNN: negkey[i,j] = 2*f_i.f_j - |f_j|^2 via one augmented PE matmul
    ([f;1]^T @ [2f;-d2]); exact top-30 per row via DVE max8/max_index/
    match_replace rounds (ranking by negkey == ranking by -dist).
  - EdgeConv MLP decomposition: relu(e@wa+ba) = relu(u_i + v_j) with
    u = x@(wa_top-wa_bot)+ba, v = x@wa_bot.  v lives in HBM row-major;
    neighbors fetched with indirect DMA (CCE-add fuses the +u_i).
  - Edge features transposed to channel-major in k-slot pairs on PE,
    relu on ACT (psum->sbuf), second MLP layer (@wb) on PE, max over
    K on DVE strided-view reduce, +bb folded into one ACT pass.
  - Head MLP computed channel-major (output transpose is free on host).
All compute fp32.
"""

import sys

sys.path.insert(0, "/opt/trn_rl_repo")

import os

import numpy as np

import concourse.bass as bass
import concourse.bacc as bacc
import concourse.mybir as mybir
import concourse.tile as tile
from concourse.bass_utils import run_bass_kernel_spmd
from concourse.masks import make_identity
from concourse import library_config

F32 = mybir.dt.float32
U32 = mybir.dt.uint32
U16 = mybir.dt.uint16
I16 = mybir.dt.int16
F16 = mybir.dt.float16
GM = os.environ.get("GM", "perk")
AF = mybir.ActivationFunctionType
ALU = mybir.AluOpType

P = 128
K = 30
KP = 32           # padded neighbor count (pad slots duplicate slot 0)
NEG_BIG = -3.0e38


def _bc_free(ap, count, inner):
    """Insert a 0-stride broadcast dim: [P, inner] -> [P, count, inner]."""
    return bass.AP(ap.tensor, ap.offset, [list(ap.ap[0]), [0, count], [1, inner]])


def _edge_conv(tc, pools, d, x_cm, wu_t, wv_t, ba_t, wb_t, bb_t, x_next,
               v_hbm, ones_t, ident, ident16, N, gsem):
    """One DynamicEdgeConv. x_cm: [d+1, N] sbuf tile (row d == 1.0).
    Writes x_next[0:64, :] (and caller-visible row 64 must be set by caller).
    """
    nc = tc.nc
    NT = N // P
    sb_work, sb_small, ps_s = pools["work"], pools["small"], pools["ps_s"]
    ps_t_pool, ps_h2_pool = pools["ps_t"], pools["ps_h2"]
    da = 32 if d <= 32 else 64   # partition row holding the aug (1 / -d2) entry

    # ---- phase A: u, v, d2, aug_r ----
    sq = sb_work.tile([64, N], F32, tag="sq", bufs=1)
    nc.scalar.activation(sq[0:d, :], x_cm[0:d, :], AF.Square)
    augr = sb_work.tile([65, N], F32, tag="augr", bufs=1)
    if d < da:
        nc.vector.memset(augr[0:da + 1, :], 0.0)
    # rows 0:d = 2*x
    nc.scalar.activation(augr[0:d, :], x_cm[0:d, :], AF.Copy, scale=2.0)
    jblk = min(512, N)
    for jc in range(N // jblk):
        ps_d2 = ps_s.tile([1, jblk], F32, tag="ps64")
        nc.tensor.matmul(ps_d2[:], ones_t[0:d, :], sq[0:d, bass.ts(jc, jblk)],
                         start=True, stop=True)
        # row da = -d2
        nc.scalar.activation(augr[da:da + 1, bass.ts(jc, jblk)], ps_d2[:],
                             AF.Copy, scale=-1.0)

    u_rm = sb_work.tile([P, NT * 64], F32, tag="u_rm", bufs=1)
    v_rm = sb_work.tile([P, NT * 64], F32, tag="v_rm", bufs=1)
    if GM == "dg":
        w_sb = sb_work.tile([P, NT * 256], I16, tag="w_sb", bufs=1, name="w_sb")
    else:
        w_sb = None
    for it in range(NT):
        ps_u = ps_s.tile([P, 64], F32, tag="ps64")
        nc.tensor.matmul(ps_u[:], x_cm[0:d, bass.ts(it, P)], wu_t[0:d, :],
                         start=True, stop=True)
        nc.vector.tensor_add(u_rm[:, bass.ts(it, 64)], ps_u[:], ba_t[:])
        ps_v = ps_s.tile([P, 64], F32, tag="ps64")
        nc.tensor.matmul(ps_v[:], x_cm[0:d, bass.ts(it, P)], wv_t[0:d, :],
                         start=True, stop=True)
        nc.scalar.activation(v_rm[:, bass.ts(it, 64)], ps_v[:], AF.Copy)
    # v -> HBM (row-major [N, 64]) as the gather source
    v_hbm_view = v_hbm[:].rearrange("(t p) c -> p t c", p=P)
    nc.sync.dma_start(v_hbm_view, v_rm[:].rearrange("p (t c) -> p t c", c=64))

    # ---- phase B: per i-tile ----
    for it in range(NT):
        # dist keys: negkey[i, j] = 2 f_i.f_j - d2_j
        negkey = sb_work.tile([P, N], F32, tag="negkey")
        for jc in range(N // jblk):
            ps_nk = ps_s.tile([P, jblk], F32, tag="ps512")
            nc.tensor.matmul(ps_nk[:], x_cm[0:da + 1, bass.ts(it, P)],
                             augr[0:da + 1, bass.ts(jc, jblk)],
                             start=True, stop=True)
            nc.scalar.activation(negkey[:, bass.ts(jc, jblk)], ps_nk[:], AF.Copy)

        # top-30 (+2 dup pads) via 4 rounds of max8
        idx_t = sb_small.tile([P, KP], U16 if GM == "dg" else U32, tag="idx")
        for r in range(4):
            m8 = sb_small.tile([P, 8], F32, tag="m8")
            nc.vector.max(m8[:], negkey[:])
            nc.vector.max_index(idx_t[:, r * 8:(r + 1) * 8], m8[:], negkey[:])
            if r < 3:
                nc.vector.match_replace(negkey[:], m8[:], negkey[:], NEG_BIG)
        # pad slots 30,31 duplicate slot 0, so max-aggregation is unaffected.
        # In perk mode they are not gathered (copied post-gather instead).
        if GM == "dg":
            nc.vector.tensor_copy(idx_t[:, 30:31], idx_t[:, 0:1])
            nc.vector.tensor_copy(idx_t[:, 31:32], idx_t[:, 0:1])

        # edge pre-activations: etile[i, k, c] = u_i[c] (+ v_j[c] via CCE add)
        etile = sb_work.tile([P, KP * 64], F32, tag="etile")
        u_slice = u_rm[:, bass.ts(it, 64)]
        if GM != "dg":
            nc.scalar.activation(etile[:], _bc_free(u_slice, KP, 64), AF.Copy)
        ev = etile[:].rearrange("p (k c) -> p k c", c=64)
        if GM == "dg":
            # build wrapped int16 index layout w[e%16, e//16] for e = k*128+i:
            # w[i%16, 8k+i//16] = idx_t[i, k], via PE transposes.
            tps = ps_s.tile([32, P], F16, tag="ps512", name="tps")
            nc.tensor.transpose(tps[:], idx_t[:].bitcast(F16), ident16[:])
            tsb = sb_small.tile([32, P], U16, tag="tsb")
            nc.vector.tensor_copy(tsb[:].bitcast(F16), tps[:])
            for q in range(8):
                tq = ps_s.tile([16, 32], F16, tag="ps512", name="tq")
                nc.tensor.transpose(tq[:], tsb[:, q * 16:(q + 1) * 16].bitcast(F16),
                                    ident16[0:32, 0:32])
                wdst = bass.AP(w_sb.tensor, w_sb[:].offset + it * 256 + q,
                               [[w_sb[:].ap[0][0], 16], [8, 32]])
                nc.vector.tensor_copy(wdst.bitcast(F16), tq[:])
            # replicate wrapped idxs to all 8 Q7 core groups (16 -> 128 rows)
            for lo, n in ((16, 16), (32, 32), (64, 64)):
                nc.sync.dma_start(w_sb[lo:lo + n, bass.ts(it, 256)],
                                  w_sb[0:n, bass.ts(it, 256)])
            with tc.tile_critical():
                nc.gpsimd.load_library(library_config.mlp)
                nc.gpsimd.dma_gather(
                    out_ap=ev, in_ap=v_hbm[:],
                    idxs_ap=w_sb[:, bass.ts(it, 256)],
                    num_idxs=P * KP, num_idxs_reg=P * KP,
                    elem_size=64, queue_num=0).then_inc(gsem[0], 16)
                gsem[1] += 16
                nc.gpsimd.wait_ge(gsem[0], gsem[1])
            nc.vector.tensor_add(etile[:], etile[:],
                                 _bc_free(u_slice, KP, 64))
        else:
            with tc.tile_critical():
                for k in range(K):
                    nc.gpsimd.indirect_dma_start(
                        out=ev[:, k, :], out_offset=None,
                        in_=v_hbm[:],
                        in_offset=bass.IndirectOffsetOnAxis(
                            ap=idx_t[:, k:k + 1], axis=0),
                        compute_op=ALU.add).then_inc(gsem[0], 16)
                gsem[1] += 16 * K
                nc.gpsimd.wait_ge(gsem[0], gsem[1])
            # pad slots 30,31 <- gathered slot 0 (exact duplicate edges)
            nc.vector.tensor_copy(ev[:, K:KP, :], _bc_free(etile[:, 0:64], 2, 64))

        # k-pair transpose -> relu -> @wb -> segment-max
        acc = None
        for g in range(KP // 8):
            ps_t = ps_t_pool.tile([P, 512], F32, tag="ps_t")
            for t in range(4):
                tt = g * 4 + t
                nc.tensor.transpose(ps_t[:, bass.ts(t, P)],
                                    etile[:, bass.ts(tt, P)], ident[:])
            rhs = sb_small.tile([P, 512], F32, tag="rhs")
            nc.scalar.activation(rhs[:], ps_t[:], AF.Relu)
            hp = ps_h2_pool.tile([64, 1024], F32, tag="ps_h2")
            nc.tensor.matmul(hp[:, 0:512], wb_t[0:64, :], rhs[0:64, :],
                             start=True, stop=True)
            nc.tensor.matmul(hp[:, 512:1024], wb_t[64:128, :], rhs[64:128, :],
                             start=True, stop=True)
            # view [64, 128 i, 2 parity, 4 t] -> reduce XY -> [64, 128]
            hview = bass.AP(hp.tensor, 0,
                            [[hp[:].ap[0][0], 64], [1, P], [512, 2], [P, 4]])
            if g == 0:
                acc = sb_small.tile([64, P], F32, tag="seg_acc")
                nc.vector.tensor_reduce(acc[:], hview, mybir.AxisListType.XY,
                                        ALU.max)
            else:
                tmp = sb_small.tile([64, P], F32, tag="seg_tmp")
                nc.vector.tensor_reduce(tmp[:], hview, mybir.AxisListType.XY,
                                        ALU.max)
                dst = (x_next[0:64, bass.ts(it, P)] if g == KP // 8 - 1
                       else acc[:])
                nc.vector.tensor_tensor(dst, acc[:], tmp[:], ALU.max)

    # +bb (per-channel bias) in place
    nc.scalar.activation(x_next[0:64, :], x_next[0:64, :], AF.Identity,
                         bias=bb_t[:, 0:1])
    # aug ones row for the next conv's lhsT
    nc.vector.memset(x_next[64:65, :], 1.0)


def _head(tc, pools, x1, x2, x3, hw1_t, hb1_t, hw2_t, hb2_t, hw3_t, hb3_t,
          out_sb, N):
    """Head MLP, channel-major: relu(feat@hw1+hb1) -> relu(@hw2+hb2) -> @hw3+hb3."""
    nc = tc.nc
    sb_head, ps_s = pools["head"], pools["ps_s"]
    xs = [x1, x2, x3]
    blk = min(512, N)
    for ib in range(N // blk):
        isl = bass.ts(ib, blk)
        h1 = sb_head.tile([P, 8 * blk], F32, tag="h1")
        for c in range(8):
            ps = ps_s.tile([P, blk], F32, tag="ps512")
            for kc in range(3):
                nc.tensor.matmul(ps[:],
                                 hw1_t[:, kc * 1024 + c * P:kc * 1024 + (c + 1) * P],
                                 xs[kc][0:64, isl],
                                 start=(kc == 0), stop=(kc == 2))
            nc.scalar.activation(h1[:, bass.ts(c, blk)], ps[:], AF.Relu,
                                 bias=hb1_t[:, c:c + 1])
        h2 = sb_head.tile([P, 4 * blk], F32, tag="h2")
        for c in range(4):
            ps = ps_s.tile([P, blk], F32, tag="ps512")
            for kc in range(8):
                nc.tensor.matmul(ps[:],
                                 hw2_t[:, kc * 512 + c * P:kc * 512 + (c + 1) * P],
                                 h1[:, bass.ts(kc, blk)],
                                 start=(kc == 0), stop=(kc == 7))
            nc.scalar.activation(h2[:, bass.ts(c, blk)], ps[:], AF.Relu,
                                 bias=hb2_t[:, c:c + 1])
        ps = ps_s.tile([P, blk], F32, tag="ps512")
        for kc in range(4):
            nc.tensor.matmul(ps[:], hw3_t[:, bass.ts(kc, P)],
                             h2[:, bass.ts(kc, blk)],
                             start=(kc == 0), stop=(kc == 3))
        nc.scalar.activation(out_sb[:, isl], ps[:], AF.Identity,
                             bias=hb3_t[:, 0:1])


def build_program(N, repeat=1):
    nc = bacc.Bacc("TRN2", target_bir_lowering=False, debug=False)
    NT = N // P

    x0cm_d = nc.dram_tensor("x0cm", [6, N], F32, kind="ExternalInput")
    w_d = {}
    for l, d in ((1, 6), (2, 64), (3, 64)):
        w_d[f"wu{l}"] = nc.dram_tensor(f"wu{l}", [d, 64], F32, kind="ExternalInput")
        w_d[f"wv{l}"] = nc.dram_tensor(f"wv{l}", [d, 64], F32, kind="ExternalInput")
        w_d[f"ba{l}"] = nc.dram_tensor(f"ba{l}", [P, 64], F32, kind="ExternalInput")
        w_d[f"wb{l}"] = nc.dram_tensor(f"wb{l}", [P, 64], F32, kind="ExternalInput")
        w_d[f"bb{l}"] = nc.dram_tensor(f"bb{l}", [64, 1], F32, kind="ExternalInput")
    w_d["hw1"] = nc.dram_tensor("hw1", [64, 3 * 1024], F32, kind="ExternalInput")
    w_d["hb1"] = nc.dram_tensor("hb1", [P, 8], F32, kind="ExternalInput")
    w_d["hw2"] = nc.dram_tensor("hw2", [P, 8 * 512], F32, kind="ExternalInput")
    w_d["hb2"] = nc.dram_tensor("hb2", [P, 4], F32, kind="ExternalInput")
    w_d["hw3"] = nc.dram_tensor("hw3", [P, 4 * P], F32, kind="ExternalInput")
    w_d["hb3"] = nc.dram_tensor("hb3", [P, 1], F32, kind="ExternalInput")
    out_d = nc.dram_tensor("out_cm", [P, N], F32, kind="ExternalOutput")
    v_hbm = [nc.dram_tensor(f"vscratch{l}", [N, 64], F32) for l in (1, 2, 3)]

    with tile.TileContext(nc) as tc:
        with tc.tile_pool(name="persist", bufs=1) as sb_p, \
             tc.tile_pool(name="work", bufs=2) as sb_work, \
             tc.tile_pool(name="small", bufs=3) as sb_small, \
             tc.tile_pool(name="head", bufs=1) as sb_head, \
             tc.tile_pool(name="ps_s", bufs=2, space="PSUM") as ps_s, \
             tc.tile_pool(name="ps_t", bufs=2, space="PSUM") as ps_t_pool, \
             tc.tile_pool(name="ps_h2", bufs=1, space="PSUM") as ps_h2_pool:
            pools = {"work": sb_work, "small": sb_small, "head": sb_head,
                     "ps_s": ps_s, "ps_t": ps_t_pool, "ps_h2": ps_h2_pool}

            ident = sb_p.tile([P, P], F32, tag="ident")
            make_identity(nc, ident[:])
            ident16 = sb_p.tile([P, P], F16, tag="ident16")
            make_identity(nc, ident16[:])
            if GM == "dg":
                with tc.tile_critical():
                    nc.gpsimd.load_library(library_config.mlp)
            ones_t = sb_p.tile([64, 1], F32, tag="ones")
            nc.vector.memset(ones_t[:], 1.0)

            def load(name, shape, tag):
                t = sb_p.tile(shape, F32, tag=tag)
                nc.sync.dma_start(t[:], w_d[name][:])
                return t

            wt = {}
            for l, d in ((1, 6), (2, 64), (3, 64)):
                wt[f"wu{l}"] = load(f"wu{l}", [d, 64], f"wu{l}")
                wt[f"wv{l}"] = load(f"wv{l}", [d, 64], f"wv{l}")
                wt[f"ba{l}"] = load(f"ba{l}", [P, 64], f"ba{l}")
                wt[f"wb{l}"] = load(f"wb{l}", [P, 64], f"wb{l}")
                wt[f"bb{l}"] = load(f"bb{l}", [64, 1], f"bb{l}")
            hw1_t = load("hw1", [64, 3 * 1024], "hw1")
            hb1_t = load("hb1", [P, 8], "hb1")
            hw2_t = load("hw2", [P, 8 * 512], "hw2")
            hb2_t = load("hb2", [P, 4], "hb2")
            hw3_t = load("hw3", [P, 4 * P], "hw3")
            hb3_t = load("hb3", [P, 1], "hb3")

            x0 = sb_p.tile([33, N], F32, tag="x0")
            nc.vector.memset(x0[:], 0.0)
            nc.sync.dma_start(x0[0:6, :], x0cm_d[:])
            nc.vector.memset(x0[32:33, :], 1.0)
            x1 = sb_p.tile([65, N], F32, tag="x1")
            x2 = sb_p.tile([65, N], F32, tag="x2")
            x3 = sb_p.tile([65, N], F32, tag="x3")

            import contextlib

            gsems = {l: nc.alloc_semaphore(f"gsem{l}") for l in (1, 2, 3)}
            out_sb = sb_p.tile([P, N], F32, tag="out_sb")

            def body():
                for l, d, xin, xout in ((1, 6, x0, x1), (2, 64, x1, x2),
                                        (3, 64, x2, x3)):
                    gsem = [gsems[l], 0]
                    _edge_conv(tc, pools, d, xin, wt[f"wu{l}"], wt[f"wv{l}"],
                               wt[f"ba{l}"], wt[f"wb{l}"], wt[f"bb{l}"], xout,
                               v_hbm[l - 1], ones_t, ident, ident16, N, gsem)
                _head(tc, pools, x1, x2, x3, hw1_t, hb1_t, hw2_t, hb2_t,
                      hw3_t, hb3_t, out_sb, N)
                nc.sync.dma_start(out_d[:], out_sb[:])

            if repeat > 1:
                with tc.For_i(0, repeat, 1):
                    body()
            else:
                body()

    nc.compile()
    return nc


def prep_weight_maps(inputs):
    """Host-side reshapes of the (shared) weights into kernel layouts."""
    f = np.asarray
    m = {}
    for l, (wa, ba, wb, bb) in ((1, ("w1a", "b1a", "w1b", "b1b")),
                                (2, ("w2a", "b2a", "w2b", "b2b")),
                                (3, ("w3a", "b3a", "w3b", "b3b"))):
        wa_np = f(inputs[wa], dtype=np.float32)
        d = wa_np.shape[0] // 2
        m[f"wu{l}"] = np.ascontiguousarray(wa_np[:d] - wa_np[d:])
        m[f"wv{l}"] = np.ascontiguousarray(wa_np[d:])
        m[f"ba{l}"] = np.ascontiguousarray(
            np.broadcast_to(f(inputs[ba], dtype=np.float32), (P, 64)))
        wb_np = f(inputs[wb], dtype=np.float32)
        m[f"wb{l}"] = np.ascontiguousarray(np.vstack([wb_np, wb_np]))
        m[f"bb{l}"] = np.ascontiguousarray(
            f(inputs[bb], dtype=np.float32).reshape(64, 1))
    hw1 = f(inputs["hw1"], dtype=np.float32)          # [192, 1024]
    m["hw1"] = np.ascontiguousarray(hw1.reshape(3, 64, 1024).transpose(1, 0, 2)
                                    .reshape(64, 3 * 1024))
    m["hb1"] = np.ascontiguousarray(
        f(inputs["hb1"], dtype=np.float32).reshape(8, P).T)
    hw2 = f(inputs["hw2"], dtype=np.float32)          # [1024, 512]
    m["hw2"] = np.ascontiguousarray(hw2.reshape(8, P, 512).transpose(1, 0, 2)
                                    .reshape(P, 8 * 512))
    m["hb2"] = np.ascontiguousarray(
        f(inputs["hb2"], dtype=np.float32).reshape(4, P).T)
    hw3 = f(inputs["hw3"], dtype=np.float32)          # [512, 128]
    m["hw3"] = np.ascontiguousarray(hw3.reshape(4, P, P).transpose(1, 0, 2)
                                    .reshape(P, 4 * P))
    m["hb3"] = np.ascontiguousarray(
        f(inputs["hb3"], dtype=np.float32).reshape(1, P).T)
    return m


_CACHE = {}


def _get_program(N):
    repeat = int(os.environ.get("REPEAT", "1"))
    key = (N, repeat)
    if key not in _CACHE:
        _CACHE[key] = build_program(N, repeat)
    return _CACHE[key]


def kernel(**inputs):
    B = int(inputs["batch_size"])
    x = np.asarray(inputs["x"], dtype=np.float32)
    pos = np.asarray(inputs["pos"], dtype=np.float32)
    N = x.shape[0] // B
    nc = _get_program(N)
    wmap = prep_weight_maps(inputs)

    x0 = np.concatenate([x, pos], axis=-1).reshape(B, N, 6)
    in_maps = []
    for b in range(B):
        im = dict(wmap)
        im["x0cm"] = np.ascontiguousarray(x0[b].T)    # [6, N]
        in_maps.append(im)

    res = run_bass_kernel_spmd(nc, in_maps, core_ids=list(range(B)))
    global LAST_RESULTS
    LAST_RESULTS = res
    out = np.stack([np.ascontiguousarray(r["out_cm"].T) for r in res.results])
    return out.astype(np.float32)


LAST_RESULTS = None


if __name__ == "__main__":
    import reference  # noqa: only for ad-hoc local testing
    ins = reference.setup_inputs()
    o = kernel(**{k: np.asarray(v) if hasattr(v, "shape") else v
                  for k, v in ins.items()})
    print(o.shape, o.dtype)
